# revision 1
# baseline (speedup 1.0000x reference)
"""CAM_Module (channel attention) Trainium2 Bass kernel, v2 (bf16).

x: (16, 512, 64, 64) f32, gamma: (1,) f32
  xf = x.reshape(B, C, N)           N = 4096
  energy = xf @ xf^T                (B, C, C)
  att = softmax(max(energy) - energy, axis=-1)   == softmax(-energy) (shift-invariant)
  out = gamma * (att @ xf) + x

Sharding: data-parallel over batch, 2 batches per core on 8 cores.

v2 design (vs v1):
  - all matmul operands bf16 (cast on SWDGE load); PSUM stays f32
  - att transpose matmul uses rhs = diag(gamma/Z) instead of identity, so
    the per-row softmax scale rides the PE transpose for free
  - residual fold: attT accumulates +I in PSUM (ident.T @ ident), so
    MM2 computes gamma*(att@xf) + x directly -> no DVE residual adds
  - PSUM->SBUF copies alternate Vector/Scalar engines
  - mirror bounce on Scalar engine (keeps DVE queue clear), f32r exact
  - tr->MM1 software pipeline with lag 4; both batches' loads up front
  - softmax chains emission-interleaved into the other batch's PE phase
"""

import sys

if "/opt/trn_rl_repo" not in sys.path:
    sys.path.insert(0, "/opt/trn_rl_repo")

from contextlib import ExitStack

import numpy as np

import concourse.bass as bass
import concourse.tile as tile
from concourse import bacc, mybir
from concourse.masks import make_identity

N_CORES = 8
B, C, H, W = 16, 512, 64, 64
N = H * W                    # 4096
BPC = B // N_CORES           # batches per core = 2
CT = C // 128                # 4 c-tiles
KT = N // 128                # 32 k-chunks (transposed layout)

F32 = mybir.dt.float32
F32R = mybir.dt.float32r
BF16 = mybir.dt.bfloat16

LAG = 4                      # tr -> MM1 pipeline depth (k-chunks)
PFX = 6                      # b1 transposes emitted before softmax(0)


def _build_nc(reps=1):
    nc = bacc.Bacc("TRN2", target_bir_lowering=False, debug=False,
                   num_devices=N_CORES)
    x_d = nc.dram_tensor("x", [BPC, C, N], F32, kind="ExternalInput").ap()
    g_d = nc.dram_tensor("gamma", [1], F32, kind="ExternalInput").ap()
    o_d = nc.dram_tensor("out", [BPC, C, N], F32, kind="ExternalOutput").ap()

    with tile.TileContext(nc) as tc, ExitStack() as ctx:
        xf_pool = ctx.enter_context(tc.tile_pool(name="xf", bufs=BPC * CT))
        xfT_pool = ctx.enter_context(tc.tile_pool(name="xfT", bufs=LAG + 4))
        att_pool = ctx.enter_context(tc.tile_pool(name="att", bufs=2 * CT))
        attT_pool = ctx.enter_context(tc.tile_pool(name="attT", bufs=2 * CT))
        d_pool = ctx.enter_context(tc.tile_pool(name="dsc", bufs=2 * CT))
        mir_pool = ctx.enter_context(tc.tile_pool(name="mir", bufs=3))
        out_pool = ctx.enter_context(tc.tile_pool(name="outp", bufs=4))
        stat_pool = ctx.enter_context(tc.tile_pool(name="stat", bufs=4 * CT))
        one_pool = ctx.enter_context(tc.tile_pool(name="one", bufs=1))
        pT = ctx.enter_context(tc.tile_pool(name="pT", bufs=2, space="PSUM"))
        pE = ctx.enter_context(tc.tile_pool(name="pE", bufs=CT, space="PSUM"))
        pO = ctx.enter_context(tc.tile_pool(name="pO", bufs=2, space="PSUM"))

        # identities for PE transpose-mode: f32 master, bf16 + f32r copies
        ident_f = one_pool.tile([128, 128], F32, tag="idf")
        make_identity(nc, ident_f[:])
        ident = one_pool.tile([128, 128], BF16, tag="idb")
        nc.vector.tensor_copy(ident[:], ident_f[:])
        ident_r = one_pool.tile([128, 128], F32R, tag="idr")
        nc.vector.tensor_copy(ident_r[:], ident_f[:])

        # broadcast gamma to all 128 partitions via K=1 matmul with ones
        g_sb = one_pool.tile([1, 1], F32, tag="gsb")
        nc.sync.dma_start(g_sb[:], g_d.rearrange("(a b) -> a b", a=1))
        ones = one_pool.tile([1, 128], F32, tag="ones")
        nc.vector.memset(ones[:], 1.0)
        pG = pT.tile([128, 1], F32, tag="pt", name="pG")
        nc.tensor.matmul(pG[:], ones[:], g_sb[:], start=True, stop=True)
        g_bc = one_pool.tile([128, 1], F32, tag="gbc")
        nc.vector.tensor_copy(g_bc[:], pG[:])

        loop_ctx = tc.For_i(0, reps, 1) if reps > 1 else None
        if loop_ctx is not None:
            ctx.enter_context(loop_ctx)

        # HAM warmup: ~2.5us of dummy matmuls fill the initial DMA wait
        # and bring the PE clock gate to 8/8 before the real transposes
        # (PE is otherwise idle-cold here; transposes alone don't count
        # as HAM activity)
        wu = pT.tile([128, 128], F32, tag="pt", name="wu")
        for i in range(24):
            nc.tensor.matmul(wu[:], ident[:], ident[:], start=True,
                             stop=True)

        # per-c-tile load chunks; chunk 0 is issued as two sub-DMAs
        # (128 + 384 cols) so the first transposes can start early.
        # SWDGE has ~1us fixed cost per dma_start, so later chunks are big.
        CHUNKS = [(0, 512), (512, 512), (1024, 1024), (2048, 1024),
                  (3072, 1024)]

        def chunk_of(col):
            for i, (off, w) in enumerate(CHUNKS):
                if off <= col < off + w:
                    return i, col - off
            raise AssertionError(col)

        st = [dict() for _ in range(BPC)]

        def emit_loads(b):
            s = st[b]
            s["xf"] = [[None] * len(CHUNKS) for _ in range(CT)]
            for q in range(len(CHUNKS)):
                off, w = CHUNKS[q]
                for ct in range(CT):
                    t = xf_pool.tile([128, w], BF16, tag=f"xf{q}",
                                     name=f"xf_{b}_{ct}_{q}")
                    if q == 0:
                        nc.gpsimd.dma_start(
                            t[:, 0:128],
                            x_d[b, ct * 128:(ct + 1) * 128, 0:128])
                        nc.gpsimd.dma_start(
                            t[:, 128:512],
                            x_d[b, ct * 128:(ct + 1) * 128, 128:512])
                    else:
                        nc.gpsimd.dma_start(
                            t[:],
                            x_d[b, ct * 128:(ct + 1) * 128, off:off + w])
                    s["xf"][ct][q] = t

        def xf_slice(b, ct, col, width):
            q, o = chunk_of(col)
            return st[b]["xf"][ct][q][:, o:o + width]

        def emit_tr(b, k):
            tp = pT.tile([128, C], BF16, tag="pt", name=f"tp_{b}_{k}")
            for ct in range(CT):
                nc.tensor.transpose(
                    tp[:, ct * 128:(ct + 1) * 128],
                    xf_slice(b, ct, k * 128, 128),
                    ident[:],
                )
            xT = xfT_pool.tile([128, C], BF16, tag="xT", name=f"xT_{b}_{k}")
            if k % 2 == 0:
                nc.vector.tensor_copy(xT[:], tp[:])
            else:
                nc.scalar.copy(xT[:], tp[:])
            return xT

        def emit_mm1(b, k, xT):
            # energy is symmetric: compute only j >= i blocks (shrinking
            # moving width per i-tile); lower blocks are mirrored after
            for it in range(CT):
                nc.tensor.matmul(
                    st[b]["e"][it][:, it * 128:C],
                    xT[:, it * 128:(it + 1) * 128],
                    xT[:, it * 128:C],
                    start=(k == 0),
                    stop=(k == KT - 1),
                )

        def emit_trmm1(b, k_from=0, prefix=(), interleave=None):
            s = st[b]
            s["e"] = [
                pE.tile([128, C], F32, tag="pe", name=f"pe_{b}_{i}")
                for i in range(CT)
            ]
            pending = list(prefix)
            for k in range(k_from, KT):
                pending.append(emit_tr(b, k))
                if len(pending) > LAG:
                    emit_mm1(b, k - len(pending) + 1, pending.pop(0))
                if interleave is not None:
                    interleave(k)
            base = KT - len(pending)
            for i, xT in enumerate(pending):
                emit_mm1(b, base + i, xT)

        def emit_mirror(b):
            # mirror lower-triangle blocks e[t][:, u] = e[u][:, t].T via
            # sbuf bounce + transpose into a scratch psum bank + ACT
            # write-back (PE never touches accumulation-grouped banks);
            # f32r keeps the mirrored energies bit-exact
            e_ps = st[b]["e"]
            for t in range(1, CT):
                mp = pT.tile([128, C], F32R, tag="pt", name=f"mp_{b}_{t}")
                for u in range(t):
                    mtmp = mir_pool.tile([128, 128], F32R, tag="mir",
                                         name=f"mir_{b}_{t}_{u}")
                    nc.scalar.copy(
                        mtmp[:], e_ps[u][:, t * 128:(t + 1) * 128])
                    nc.tensor.transpose(
                        mp[:, u * 128:(u + 1) * 128], mtmp[:], ident_r[:])
                nc.scalar.copy(
                    e_ps[t][:, 0:t * 128], mp[:, 0:t * 128])

        def emit_softmax(b, it):
            # att row i = exp(min_i - e_i) * gamma / Z_i; the 1/Z*gamma
            # scale is deferred to the attT transpose via D = diag(rz*g)
            s = st[b]
            if it == 0:
                s["att"] = [None] * CT
                s["D"] = [None] * CT
            m = stat_pool.tile([128, 1], F32, tag="m", name=f"m_{b}_{it}")
            nc.vector.tensor_reduce(
                m[:], s["e"][it][:], axis=mybir.AxisListType.X,
                op=mybir.AluOpType.min,
            )
            a = att_pool.tile([128, C], BF16, tag="a", name=f"a_{b}_{it}")
            z = stat_pool.tile([128, 1], F32, tag="z", name=f"z_{b}_{it}")
            nc.scalar.activation(
                a[:], s["e"][it][:], mybir.ActivationFunctionType.Exp,
                bias=m[:], scale=-1.0, accum_out=z[:],
            )
            rz = stat_pool.tile([128, 1], F32, tag="rz", name=f"rz_{b}_{it}")
            nc.vector.reciprocal(rz[:], z[:])
            g = stat_pool.tile([128, 1], F32, tag="g", name=f"g_{b}_{it}")
            nc.vector.tensor_mul(g[:], rz[:], g_bc[:])
            D = d_pool.tile([128, 128], BF16, tag="D", name=f"D_{b}_{it}")
            nc.vector.tensor_scalar_mul(D[:], ident[:], g[:])
            s["att"][it] = a
            s["D"][it] = D

        def emit_attT(b, jts=None):
            # aT[jt] = (att.T * colscale)[:, :] + I  built in PSUM:
            #   per it: E[it][:, jt].T @ D[it]  (transpose + row scale)
            #   then ident.T @ ident accumulated into the diagonal block
            s = st[b]
            if "attT" not in s:
                s["attT"] = []
            for jt in jts if jts is not None else range(CT):
                tp = pT.tile([128, C], F32, tag="pt", name=f"at_{b}_{jt}")
                for it in range(CT):
                    nc.tensor.matmul(
                        tp[:, it * 128:(it + 1) * 128],
                        s["att"][it][:, jt * 128:(jt + 1) * 128],
                        s["D"][it][:],
                        start=True,
                        stop=(it != jt),
                    )
                    if it == jt:
                        nc.tensor.matmul(
                            tp[:, it * 128:(it + 1) * 128],
                            ident[:], ident[:],
                            start=False, stop=True,
                        )
                aT = attT_pool.tile([128, C], BF16, tag="aT",
                                    name=f"aT_{b}_{jt}")
                if jt % 2 == 0:
                    nc.vector.tensor_copy(aT[:], tp[:])
                else:
                    nc.scalar.copy(aT[:], tp[:])
                s["attT"].append(aT)

        def emit_mm2_chunk(b, it, nch, ci, only_po=False):
            # out[it, nch] = sum_jt aT[jt][:, it].T @ xf[jt, nch]
            #             == gamma*(att@xf) + x   (identity is inside aT)
            # rotate over 4 PSUM banks (pO's 2 + pT's 2) so matmuls never
            # wait on copy latency; only_po while pT is busy with tr tiles
            s = st[b]
            pool = pO if (only_po or (ci % 4) < 2) else pT
            po = pool.tile([128, 512], F32,
                           tag="po" if pool is pO else "pt",
                           name=f"po_{b}_{it}_{nch}")
            for jt in range(CT):
                nc.tensor.matmul(
                    po[:],
                    s["attT"][jt][:, it * 128:(it + 1) * 128],
                    xf_slice(b, jt, nch * 512, 512),
                    start=(jt == 0),
                    stop=(jt == CT - 1),
                )
            o_t = out_pool.tile([128, 512], F32, tag="o",
                                name=f"o_{b}_{it}_{nch}")
            if ci % 2 == 0:
                nc.scalar.copy(o_t[:], po[:])
            else:
                nc.vector.tensor_copy(o_t[:], po[:])
            nc.sync.dma_start(
                o_d[b, it * 128:(it + 1) * 128,
                    nch * 512:(nch + 1) * 512],
                o_t[:],
            )

        def emit_mm2(b, chunks=None, interleave=None, only_po=False):
            s = st[b]
            for ci, (it, nch) in enumerate(
                    chunks if chunks is not None else
                    [(i, n) for i in range(CT) for n in range(N // 512)]):
                emit_mm2_chunk(b, it, nch, ci, only_po=only_po)
                if interleave is not None:
                    interleave(ci)

        # ---- emission schedule ----
        emit_loads(0)
        emit_loads(1)
        emit_trmm1(0)
        emit_mirror(0)
        emit_softmax(0, 0)  # e[0] needs no mirror; unblocks b1's MM1 early
        pfx = [emit_tr(1, k) for k in range(PFX)]

        ALL_CHUNKS = [(i, n) for i in range(CT) for n in range(N // 512)]

        def ilv_sm0(k):
            # softmax(0), attT(0), then the first mm2(0) chunks ride
            # inside b1's tr+MM1 phase: PE has buffered mm2 work to chew
            # while the MM1 tail waits on b1's final (DMA-starved) loads,
            # and the output store stream starts the moment loads end
            if PFX + 1 <= k <= PFX + 3:
                emit_softmax(0, k - PFX)
            elif 16 <= k <= 22 and k % 2 == 0:
                emit_attT(0, jts=[(k - 16) // 2])
            elif k >= 24:
                it, nch = ALL_CHUNKS[k - 24]
                emit_mm2_chunk(0, it, nch, ci=k, only_po=True)

        emit_trmm1(1, k_from=PFX, prefix=pfx, interleave=ilv_sm0)
        emit_mirror(1)

        def ilv_sm1(ci):
            # softmax(1) and attT(1) ride inside b0's MM2 phase
            if 1 <= ci <= 4:
                emit_softmax(1, ci - 1)
            elif 14 <= ci <= 20 and ci % 2 == 0:
                emit_attT(1, jts=[(ci - 14) // 2])

        emit_mm2(0, chunks=ALL_CHUNKS[8:], interleave=ilv_sm1)
        emit_mm2(1)

    nc.compile()
    return nc


_RUNNER = None


def _build_runner(nc=None):
    """Compile once; return a callable (xf_full, gamma) -> out_full.

    Mirrors concourse.bass2jax.run_bass_via_pjrt but caches the jitted
    shard_map executable so repeated kernel() calls don't re-lower, and
    keeps the output-seed zero buffers resident on device.
    """
    import jax
    from jax.sharding import Mesh, NamedSharding, PartitionSpec
    from jax.experimental.shard_map import shard_map

    from concourse import bass2jax, mybir as _mybir
    from concourse.bass2jax import _bass_exec_p, partition_id_tensor

    if nc is None:
        nc = _build_nc()
    bass2jax.install_neuronx_cc_hook()

    partition_name = (
        nc.partition_id_tensor.name if nc.partition_id_tensor else None
    )
    in_names, out_names, out_avals, zero_shapes = [], [], [], []
    for alloc in nc.m.functions[0].allocations:
        if not isinstance(alloc, _mybir.MemoryLocationSet):
            continue
        name = alloc.memorylocations[0].name
        if alloc.kind == "ExternalInput":
            if name != partition_name:
                in_names.append(name)
        elif alloc.kind == "ExternalOutput":
            shape = tuple(alloc.tensor_shape)
            dtype = _mybir.dt.np(alloc.dtype)
            out_names.append(name)
            out_avals.append(jax.core.ShapedArray(shape, dtype))
            zero_shapes.append((shape, dtype))
    n_params = len(in_names)
    all_names = list(in_names) + list(out_names)
    if partition_name is not None:
        all_names.append(partition_name)

    def _body(*args):
        operands = list(args)
        if partition_name is not None:
            operands.append(partition_id_tensor())
        return tuple(
            _bass_exec_p.bind(
                *operands,
                out_avals=tuple(out_avals),
                in_names=tuple(all_names),
                out_names=tuple(out_names),
                lowering_input_output_aliases=(),
                sim_require_finite=True,
                sim_require_nnan=True,
                nc=nc,
            )
        )

    devices = jax.devices()[:N_CORES]
    mesh = Mesh(np.asarray(devices), ("core",))
    n_in = n_params + len(out_names)
    sharded = jax.jit(
        shard_map(
            _body,
            mesh=mesh,
            in_specs=(PartitionSpec("core"),) * n_in,
            out_specs=(PartitionSpec("core"),) * len(out_names),
            check_rep=False,
        ),
        keep_unused=True,
    )

    # in_names order is discovered from allocations; map our two inputs
    assert set(in_names) == {"x", "gamma"}, in_names

    # output-seed buffers created on device once (kernel writes out fully)
    sh = NamedSharding(mesh, PartitionSpec("core"))
    zeros_dev = [
        jax.jit(
            lambda s=s, d=d: jax.numpy.zeros((N_CORES * s[0],) + s[1:], d),
            out_shardings=sh,
        )()
        for s, d in zero_shapes
    ]
    jax.block_until_ready(zeros_dev)

    def run(xf_full, gamma):
        per_in = {
            "x": xf_full,  # (16, 512, 4096) == concat of per-core (2, 512, 4096)
            "gamma": np.ascontiguousarray(
                np.broadcast_to(np.asarray(gamma, np.float32).reshape(1),
                                (N_CORES,))
            ),
        }
        concat_in = [per_in[name] for name in in_names]
        out_arrs = sharded(*concat_in, *zeros_dev)
        return np.asarray(out_arrs[out_names.index("out")])

    run.sharded = sharded
    run.zeros_dev = zeros_dev
    run.in_names = in_names
    run.out_names = out_names
    run.mesh = mesh
    return run


def _get_runner():
    global _RUNNER
    if _RUNNER is None:
        _RUNNER = _build_runner()
    return _RUNNER


def kernel(x, gamma):
    assert x.shape == (B, C, H, W)
    run = _get_runner()
    xf = np.ascontiguousarray(np.asarray(x, np.float32).reshape(B, C, N))
    g = np.asarray(gamma, np.float32)
    out = run(xf, g)
    return out.reshape(B, C, H, W).astype(np.float32, copy=False)

